# revision 1
# baseline (speedup 1.0000x reference)
"""Hamiltonian operator kernel for 8 Trainium2 NeuronCores.

Sharding (per spec hint): data-parallel over batch B=4 across 4 device
groups x tensor-parallel factor 2 within each group (attention heads
4+4, FFN hidden 2048+2048, semantic-metric score rows 1024+1024).
Collectives: psum over the tensor axis for attention out-proj, FFN
second matmul, and the attention-entropy reduction.
"""
import numpy as np
import jax
import jax.numpy as jnp
from jax.sharding import PartitionSpec as P
from functools import partial

B, T, DIM, H = 4, 2048, 1024, 8
HD = DIM // H
GAMMA = 0.05
DT_MIN, DT_MAX = 0.05, 0.3
TP = 2            # tensor-parallel factor
DP = 4            # data-parallel factor (over batch)
DH = DIM // TP    # per-device head-group width (4 heads * 128)
FH = 4 * DIM // TP  # per-device FFN hidden width
TR = T // TP      # per-device score rows

_compiled = None


def _layernorm(x, g, b, eps=1e-5):
    mu = x.mean(-1, keepdims=True)
    var = ((x - mu) ** 2).mean(-1, keepdims=True)
    return (x - mu) * jax.lax.rsqrt(var + eps) * g + b


def _shard_fn(state, u, mass_w, ln1_g, ln1_b, in_proj_w, in_proj_b,
              out_proj_w, out_proj_b, ln2_g, ln2_b, ff_w1, ff_b1,
              ff_w2, ff_b2):
    # state/u: [1, T, 2*DIM] (batch-sharded); weights replicated.
    ti = jax.lax.axis_index('t')
    q, p = jnp.split(state, 2, axis=-1)
    u_q, u_p = jnp.split(u, 2, axis=-1)

    # ---- semantic metric ----
    z = q
    z_shifted = jnp.roll(z, 1, axis=1)
    local_variance = ((z - z_shifted) ** 2).mean(-1)                  # [1,T]
    zn = jnp.maximum(jnp.linalg.norm(z, axis=-1), 1e-8)
    un = jnp.maximum(jnp.linalg.norm(u_q, axis=-1), 1e-8)
    alignment = (z * u_q).sum(-1) / (zn * un)
    difficulty = (1.0 - alignment) / 2.0

    # score rows split over the tensor axis: [1, TR, T]
    z_rows = jax.lax.dynamic_slice_in_dim(z, ti * TR, TR, axis=1)
    scores = jnp.einsum('btd,bsd->bts', z_rows, z) / jnp.sqrt(jnp.float32(DIM))
    w = jax.nn.softmax(scores, axis=-1)
    ent_local = -(w * jnp.log(w + 1e-10)).sum(-1).sum(-1)             # [1]
    entropy = jax.lax.psum(ent_local, 't') / T                        # [1]

    variance_norm = jax.nn.sigmoid(local_variance * 10.0)
    entropy_norm = entropy[:, None] / 10.0
    complexity = jnp.clip(variance_norm * difficulty * entropy_norm, 0.0, 1.0)
    dt = (DT_MIN + (DT_MAX - DT_MIN) * (1.0 - complexity))[..., None]  # [1,T,1]

    # ---- attention (heads split TP-ways) ----
    x = _layernorm(q + u_q, ln1_g, ln1_b)
    w_q = jax.lax.dynamic_slice_in_dim(in_proj_w[0:DIM], ti * DH, DH, 0)
    w_k = jax.lax.dynamic_slice_in_dim(in_proj_w[DIM:2 * DIM], ti * DH, DH, 0)
    w_v = jax.lax.dynamic_slice_in_dim(in_proj_w[2 * DIM:], ti * DH, DH, 0)
    b_q = jax.lax.dynamic_slice_in_dim(in_proj_b[0:DIM], ti * DH, DH, 0)
    b_k = jax.lax.dynamic_slice_in_dim(in_proj_b[DIM:2 * DIM], ti * DH, DH, 0)
    b_v = jax.lax.dynamic_slice_in_dim(in_proj_b[2 * DIM:], ti * DH, DH, 0)
    HL = H // TP
    qh = (x @ w_q.T + b_q).reshape(1, T, HL, HD)
    kh = (x @ w_k.T + b_k).reshape(1, T, HL, HD)
    vh = (x @ w_v.T + b_v).reshape(1, T, HL, HD)
    att = jnp.einsum('bthd,bshd->bhts', qh, kh) / jnp.sqrt(jnp.float32(HD))
    att = jax.nn.softmax(att, axis=-1)
    o = jnp.einsum('bhts,bshd->bthd', att, vh).reshape(1, T, DH)
    w_o = jax.lax.dynamic_slice_in_dim(out_proj_w, ti * DH, DH, 1)
    attn_out = jax.lax.psum(o @ w_o.T, 't') + out_proj_b

    # ---- FFN (hidden split TP-ways) ----
    h = _layernorm(q + attn_out, ln2_g, ln2_b)
    w1 = jax.lax.dynamic_slice_in_dim(ff_w1, ti * FH, FH, 0)
    b1 = jax.lax.dynamic_slice_in_dim(ff_b1, ti * FH, FH, 0)
    w2 = jax.lax.dynamic_slice_in_dim(ff_w2, ti * FH, FH, 1)
    hid = jax.nn.gelu(h @ w1.T + b1, approximate=False)
    force = jax.lax.psum(hid @ w2.T, 't') + ff_b2

    # ---- symplectic Euler ----
    p_new = p + dt * (force - GAMMA * p)
    velocity = (p_new + u_p) @ mass_w.T
    q_new = q + dt * velocity
    out = jnp.concatenate([q_new, p_new], axis=-1)                    # [1,T,4096]
    # return this device's half of the T rows so the output is T-sharded
    return jax.lax.dynamic_slice_in_dim(out, ti * TR, TR, axis=1)


def _build():
    mesh = jax.make_mesh((DP, TP), ('b', 't'))
    rep2 = P(None, None)
    rep1 = P(None)
    in_specs = (P('b', None, None), P('b', None, None),
                rep2, rep1, rep1, rep2, rep1, rep2, rep1,
                rep1, rep1, rep2, rep1, rep2, rep1)
    fn = jax.shard_map(_shard_fn, mesh=mesh, in_specs=in_specs,
                       out_specs=P('b', 't', None), check_vma=False)
    return jax.jit(fn)


def kernel(**inputs):
    global _compiled
    if _compiled is None:
        _compiled = _build()
    order = ['state', 'u', 'mass_w', 'ln1_g', 'ln1_b', 'in_proj_w',
             'in_proj_b', 'out_proj_w', 'out_proj_b', 'ln2_g', 'ln2_b',
             'ff_w1', 'ff_b1', 'ff_w2', 'ff_b2']
    args = [np.asarray(inputs[k], dtype=np.float32) for k in order]
    out = _compiled(*args)
    return np.asarray(jax.device_get(out), dtype=np.float32)


# revision 5
# speedup vs baseline: 1.0208x; 1.0208x over previous
"""Hamiltonian operator kernel for 8 Trainium2 NeuronCores.

Sharding (per spec hint): data-parallel over batch B=4 across 4 device
groups x tensor-parallel factor 2 within each group (attention heads
4+4, FFN hidden 2048+2048, semantic-metric score rows 1024+1024).
Collectives: psum over the tensor axis for attention out-proj, FFN
second matmul, and the attention-entropy reduction.
"""
import numpy as np
import jax
import jax.numpy as jnp
from jax.sharding import PartitionSpec as P
from functools import partial

B, T, DIM, H = 4, 2048, 1024, 8
HD = DIM // H
GAMMA = 0.05
DT_MIN, DT_MAX = 0.05, 0.3
TP = 2            # tensor-parallel factor
DP = 4            # data-parallel factor (over batch)
DH = DIM // TP    # per-device head-group width (4 heads * 128)
FH = 4 * DIM // TP  # per-device FFN hidden width
TR = T // TP      # per-device score rows

_compiled = None


def _layernorm(x, g, b, eps=1e-5):
    mu = x.mean(-1, keepdims=True)
    var = ((x - mu) ** 2).mean(-1, keepdims=True)
    return (x - mu) * jax.lax.rsqrt(var + eps) * g + b


def _shard_fn(state, u, mass_w, ln1_g, ln1_b, in_proj_w, in_proj_b,
              out_proj_w, out_proj_b, ln2_g, ln2_b, ff_w1, ff_b1,
              ff_w2, ff_b2):
    # state/u: [1, T, 2*DIM] (batch-sharded); weights replicated.
    ti = jax.lax.axis_index('t')
    q, p = jnp.split(state, 2, axis=-1)
    u_q, u_p = jnp.split(u, 2, axis=-1)

    # ---- semantic metric ----
    z = q
    z_shifted = jnp.roll(z, 1, axis=1)
    local_variance = ((z - z_shifted) ** 2).mean(-1)                  # [1,T]
    zn = jnp.maximum(jnp.linalg.norm(z, axis=-1), 1e-8)
    un = jnp.maximum(jnp.linalg.norm(u_q, axis=-1), 1e-8)
    alignment = (z * u_q).sum(-1) / (zn * un)
    difficulty = (1.0 - alignment) / 2.0

    # score rows split over the tensor axis: [1, TR, T]
    z_rows = jax.lax.dynamic_slice_in_dim(z, ti * TR, TR, axis=1)
    scores = jnp.einsum('btd,bsd->bts', z_rows, z) / jnp.sqrt(jnp.float32(DIM))
    w = jax.nn.softmax(scores, axis=-1)
    ent_local = -(w * jnp.log(w + 1e-10)).sum(-1).sum(-1)             # [1]
    entropy = jax.lax.psum(ent_local, 't') / T                        # [1]

    variance_norm = jax.nn.sigmoid(local_variance * 10.0)
    entropy_norm = entropy[:, None] / 10.0
    complexity = jnp.clip(variance_norm * difficulty * entropy_norm, 0.0, 1.0)
    dt = (DT_MIN + (DT_MAX - DT_MIN) * (1.0 - complexity))[..., None]  # [1,T,1]

    # ---- attention (heads split TP-ways) ----
    x = _layernorm(q + u_q, ln1_g, ln1_b)
    w_q = jax.lax.dynamic_slice_in_dim(in_proj_w[0:DIM], ti * DH, DH, 0)
    w_k = jax.lax.dynamic_slice_in_dim(in_proj_w[DIM:2 * DIM], ti * DH, DH, 0)
    w_v = jax.lax.dynamic_slice_in_dim(in_proj_w[2 * DIM:], ti * DH, DH, 0)
    b_q = jax.lax.dynamic_slice_in_dim(in_proj_b[0:DIM], ti * DH, DH, 0)
    b_k = jax.lax.dynamic_slice_in_dim(in_proj_b[DIM:2 * DIM], ti * DH, DH, 0)
    b_v = jax.lax.dynamic_slice_in_dim(in_proj_b[2 * DIM:], ti * DH, DH, 0)
    HL = H // TP
    qh = (x @ w_q.T + b_q).reshape(1, T, HL, HD)
    kh = (x @ w_k.T + b_k).reshape(1, T, HL, HD)
    vh = (x @ w_v.T + b_v).reshape(1, T, HL, HD)
    att = jnp.einsum('bthd,bshd->bhts', qh, kh) / jnp.sqrt(jnp.float32(HD))
    att = jax.nn.softmax(att, axis=-1)
    o = jnp.einsum('bhts,bshd->bthd', att, vh).reshape(1, T, DH)
    w_o = jax.lax.dynamic_slice_in_dim(out_proj_w, ti * DH, DH, 1)
    attn_out = jax.lax.psum(o @ w_o.T, 't') + out_proj_b

    # ---- FFN (hidden split TP-ways) ----
    h = _layernorm(q + attn_out, ln2_g, ln2_b)
    w1 = jax.lax.dynamic_slice_in_dim(ff_w1, ti * FH, FH, 0)
    b1 = jax.lax.dynamic_slice_in_dim(ff_b1, ti * FH, FH, 0)
    w2 = jax.lax.dynamic_slice_in_dim(ff_w2, ti * FH, FH, 1)
    hid = jax.nn.gelu(h @ w1.T + b1, approximate=False)
    force = jax.lax.psum(hid @ w2.T, 't') + ff_b2

    # ---- symplectic Euler ----
    p_new = p + dt * (force - GAMMA * p)
    velocity = (p_new + u_p) @ mass_w.T
    q_new = q + dt * velocity
    out = jnp.concatenate([q_new, p_new], axis=-1)                    # [1,T,4096]
    # return this device's half of the T rows so the output is T-sharded
    return jax.lax.dynamic_slice_in_dim(out, ti * TR, TR, axis=1)


def _build():
    mesh = jax.make_mesh((DP, TP), ('b', 't'))
    rep2 = P(None, None)
    rep1 = P(None)
    in_specs = (P('b', None, None), P('b', None, None),
                rep2, rep1, rep1, rep2, rep1, rep2, rep1,
                rep1, rep1, rep2, rep1, rep2, rep1)
    fn = jax.shard_map(_shard_fn, mesh=mesh, in_specs=in_specs,
                       out_specs=P('b', 't', None), check_vma=False)
    return jax.jit(fn)


def kernel(**inputs):
    global _compiled
    if _compiled is None:
        _compiled = _build()
    order = ['state', 'u', 'mass_w', 'ln1_g', 'ln1_b', 'in_proj_w',
             'in_proj_b', 'out_proj_w', 'out_proj_b', 'ln2_g', 'ln2_b',
             'ff_w1', 'ff_b1', 'ff_w2', 'ff_b2']
    args = [np.asarray(inputs[k], dtype=np.float32) for k in order]
    out = _compiled(*args)
    return np.asarray(jax.device_get(out), dtype=np.float32)
